# revision 6
# baseline (speedup 1.0000x reference)
"""ALIF spiking-network forward + eligibility traces on 8 Trainium2 NeuronCores.

Data-parallel: batch 32 sharded 4 samples/core. The only sequential part is the
(B,H) spike recurrence (48 steps); the heavy per-sample eligibility-trace
matrices (eps/fe, H x I and H x H) are collapsed algebraically into per-(h,t)
scalars + a few small matmuls:

  eps_t = (rho - beta*psi_t) eps_{t-1} + psi_t (x) tr_t   (rank-1 updates, row decay)
  fe_T  = sum_t kappa^{T-t} psi_t (x) tr_t - beta * ...
        = sum_s W_s (x) tr_s,  W_s = psi_s (kap_s - beta R_s),
  R_s   = kap_s psi_s + c_{s+1} R_{s+1}          (backward linear scan)
  tr    = La @ X   (triangular alpha filter, absorbed as a matmul)

so  fe_in = W^T @ (La @ X),  fe_rec = W^T @ (La'' @ Z).
"""

import math

import numpy as np

# ---------------------------------------------------------------- constants
ALPHA = float(np.exp(-1.0 / 20.0))
RHO = float(np.exp(-1.0 / 2000.0))
KAPPA = float(np.exp(-1.0 / 20.0))
BETA = 0.07
VTH = 0.6
GAMMA = 0.3

B, T, I, H, O = 32, 48, 128, 256, 64
NCORES = 8
BS = B // NCORES          # 4 samples per core
HT = H // 128             # 2 h-tiles
FW = HT * BS              # 8 = free width of loop state tiles

C1_ = VTH * (1.0 - RHO)
W_ = 1.0 / ALPHA - 1.0 / RHO
U_ = C1_ / RHO
K1_ = W_ * (BETA + C1_) + U_ * (1.0 - RHO)
K2_ = W_ * C1_ + U_ * (1.0 - RHO)
M1_ = 1.0 / (ALPHA * W_)
N1_ = (VTH - U_ / W_) / ALPHA
Q0_ = VTH / ALPHA
B0_ = W_ * VTH + U_

_OPS_REGISTERED = {}


def _register_dve_ops():
    """Two fused DVE ops for the loop state updates (idempotent)."""
    global _OPS_REGISTERED
    if _OPS_REGISTERED:
        return _OPS_REGISTERED
    import concourse.dve_ops as dve_ops
    from concourse.dve_ops import OPS, DveOp
    from concourse.dve_spec import Spec, Src0, Src1, C0, C1, C2, select, Zero, lower
    from concourse.dve_uop import DveOpSpec

    def mk(name, spec):
        for o in OPS:
            if o.name == name:
                return o
        shas = {}
        for ver in ("v3", "v4"):
            s = DveOpSpec(name=name, opcode=0, uops=lower(spec, ver=ver), rd1_en=True)
            shas[ver] = s.sha(ver)
        op = DveOp(name, spec, subdim=False, uops_sha=shas)
        OPS.append(op)
        dve_ops.CUSTOM_DVE_SPECS[name] = spec
        dve_ops._SUB_OPCODE_FOR_NAME[name] = dve_ops._CUSTOM_DVE_ROW_BASE + len(OPS) - 1
        assert dve_ops._SUB_OPCODE_FOR_NAME[name] < 0x20
        return op

    # B' = rho*B + (d>0 ? k1 : k2)      (B = affine-transformed threshold state)
    aop = mk(
        "ALIF_BUPD",
        Spec(
            body=Src1 * C0 + select(Src0 > Zero, C1, C2),
            reference=lambda in0, in1, s0, s1, imm2: (
                np.asarray(in1) * s0 + np.where(np.asarray(in0) > 0, s1, imm2)
            ).astype(np.float32),
        ),
    )
    # q' = (d>0 ? m1*B' + n1 : B') - d   (q: next-step threshold, d_{t+1}=i-alpha*q)
    qop = mk(
        "ALIF_QUPD",
        Spec(
            body=select(Src0 > Zero, C0 * Src1 + C1, Src1) - Src0,
            reference=lambda in0, in1, s0, s1, imm2: (
                np.where(np.asarray(in0) > 0, s0 * np.asarray(in1) + s1, np.asarray(in1))
                - np.asarray(in0)
            ).astype(np.float32),
        ),
    )
    _OPS_REGISTERED = {"AOP": aop, "QOP": qop}
    return _OPS_REGISTERED


def _const_arrays():
    t = np.arange(T)
    # 0.5-scaled triangular alpha filters (absorb psi's gamma/vth=0.5 factor)
    laT = np.where(t[None, :] >= t[:, None], 0.5 * ALPHA ** (t[None, :] - t[:, None]), 0.0)
    la2T = np.where(t[None, :] - 1 >= t[:, None], 0.5 * ALPHA ** (t[None, :] - 1 - t[:, None]), 0.0)
    kap = np.zeros((128, T * FW), np.float32)
    for tt in range(T):
        kap[:, FW * tt : FW * (tt + 1)] = KAPPA ** (T - 1 - tt)
    ident = np.eye(128, dtype=np.float32)
    return laT.astype(np.float32), la2T.astype(np.float32), kap, ident


def build(nc, debug=False):
    """Build the per-core SPMD graph (same on all 8 cores)."""
    import concourse.mybir as mybir
    from concourse.tile import TileContext

    ops = _register_dve_ops()
    AOP, QOP = ops["AOP"], ops["QOP"]
    f32 = mybir.dt.float32
    Alu = mybir.AluOpType
    Act = mybir.ActivationFunctionType

    laT_np, la2T_np, kap_np, ident_np = _const_arrays()

    # ---- DRAM I/O -------------------------------------------------------
    x_itb_d = nc.dram_tensor("x_itb", [I, T * BS], f32, kind="ExternalInput")
    x_tbi_d = nc.dram_tensor("x_tbi", [T, BS * I], f32, kind="ExternalInput")
    w_inT_d = nc.dram_tensor("w_inT", [I, H], f32, kind="ExternalInput")
    w_recT_d = nc.dram_tensor("w_recT2", [128, HT * H], f32, kind="ExternalInput")
    w_outT_d = nc.dram_tensor("w_outT2", [128, HT * O], f32, kind="ExternalInput")
    o_fe_d = nc.dram_tensor("o_fe", [128, BS * HT * (I + H)], f32, kind="ExternalOutput")
    o_ro_d = nc.dram_tensor("o_ro", [O, BS * T], f32, kind="ExternalOutput")
    if debug:
        dbg_z_d = nc.dram_tensor("dbg_z", [128, T * FW], f32, kind="ExternalOutput")
        dbg_d_d = nc.dram_tensor("dbg_d", [128, T * FW], f32, kind="ExternalOutput")
        dbg_w_d = nc.dram_tensor("dbg_w", [128, T * FW], f32, kind="ExternalOutput")

    laT_c = nc.inline_tensor(laT_np, name="laT")
    la2T_c = nc.inline_tensor(la2T_np, name="la2T")
    kap_c = nc.inline_tensor(kap_np, name="kapc")
    ident_c = nc.inline_tensor(ident_np, name="identc")

    with TileContext(nc) as tc:
        with (
            tc.tile_pool(name="sb", bufs=1) as sb,
            tc.tile_pool(name="ps_i", bufs=1, space="PSUM") as ps_i,
            tc.tile_pool(name="ps_tr", bufs=2, space="PSUM") as ps_tr,
            tc.tile_pool(name="ps_y", bufs=1, space="PSUM") as ps_y,
            tc.tile_pool(name="ps_tp", bufs=2, space="PSUM") as ps_tp,
            tc.tile_pool(name="ps_fe", bufs=2, space="PSUM") as ps_fe,
        ):
            # ---- load inputs / constants to SBUF -----------------------
            x_itb = sb.tile([I, T * BS], f32, tag="x_itb")
            x_tbi = sb.tile([T, BS * I], f32, tag="x_tbi")
            w_inT = sb.tile([I, H], f32, tag="w_inT")
            wrec = sb.tile([128, HT * H], f32, tag="wrec")
            woutT = sb.tile([128, HT * O], f32, tag="woutT")
            laT = sb.tile([T, T], f32, tag="laT")
            la2T = sb.tile([T, T], f32, tag="la2T")
            kap = sb.tile([128, T * FW], f32, tag="kap")
            ident = sb.tile([128, 128], f32, tag="ident")
            nc.sync.dma_start(x_itb[:], x_itb_d.ap())
            nc.sync.dma_start(x_tbi[:], x_tbi_d.ap())
            nc.sync.dma_start(w_inT[:], w_inT_d.ap())
            nc.sync.dma_start(wrec[:], w_recT_d.ap())
            nc.sync.dma_start(woutT[:], w_outT_d.ap())
            nc.sync.dma_start(laT[:], laT_c.ap())
            nc.sync.dma_start(la2T[:], la2T_c.ap())
            nc.sync.dma_start(kap[:], kap_c.ap())
            nc.sync.dma_start(ident[:], ident_c.ap())

            # ---- state + history buffers -------------------------------
            q = sb.tile([128, FW], f32, tag="q")
            Bst = sb.tile([128, FW], f32, tag="Bst")
            Zbuf = sb.tile([128, T * FW], f32, tag="Zbuf")     # z_t at col FW*t+4*ht+b
            Dbuf = sb.tile([128, T * FW], f32, tag="Dbuf")     # d_t
            nc.gpsimd.memset(q[:], Q0_)
            nc.gpsimd.memset(Bst[:], B0_)

            # ---- I_bank[h, ht*192 + 4t + b] = i_t (x-proj + recurrent) -
            I_bank = ps_i.tile([128, HT * T * BS], f32, tag="I_bank")  # 384 cols, 1 bank

            # ---- TrIn = (0.5 La) @ X  (early; independent of loop) -----
            trin_ps = ps_tr.tile([T, 512], f32, tag="tr_ps")
            nc.tensor.matmul(trin_ps[:, : BS * I], lhsT=laT[:], rhs=x_tbi[:], start=True, stop=True)
            TrIn = sb.tile([T, BS * I], f32, tag="TrIn")
            nc.scalar.copy(TrIn[:], trin_ps[:, : BS * I])

            i_v = I_bank[:].rearrange("p (h t b) -> p h t b", h=HT, b=BS)

            # ---- the spike recurrence ----------------------------------
            for t in range(T):
                for ht in range(HT):
                    nc.tensor.matmul(
                        i_v[:, ht, t, :],
                        lhsT=w_inT[:, ht * 128 : (ht + 1) * 128],
                        rhs=x_itb[:, BS * t : BS * (t + 1)],
                        start=True,
                        stop=(t == 0),
                    )
                    if t > 0:
                        for kt in range(HT):
                            nc.tensor.matmul(
                                i_v[:, ht, t, :],
                                lhsT=wrec[:, kt * H + ht * 128 : kt * H + (ht + 1) * 128],
                                rhs=Zbuf[:, FW * (t - 1) + BS * kt : FW * (t - 1) + BS * (kt + 1)],
                                start=False,
                                stop=(kt == HT - 1),
                            )
                # d_t = i_t - alpha * q   -> Dbuf
                nc.vector.scalar_tensor_tensor(
                    Dbuf[:].rearrange("p (t x) -> p t x", x=FW)[:, t, :],
                    in0=q[:],
                    scalar=-ALPHA,
                    in1=i_v[:, :, t, :],
                    op0=Alu.mult,
                    op1=Alu.add,
                )
                dsl = Dbuf[:, FW * t : FW * (t + 1)]
                # z_t = d_t > 0          -> Zbuf
                nc.vector.tensor_scalar(
                    Zbuf[:, FW * t : FW * (t + 1)], dsl, 0.0, None, Alu.is_gt
                )
                # B' = rho*B + (z ? k1 : k2)
                nc.vector._custom_dve(AOP, out=Bst[:], in0=dsl, in1=Bst[:], s0=RHO, s1=K1_, imm2=K2_)
                # q' = (z ? m1*B' + n1 : B') - d
                nc.vector._custom_dve(QOP, out=q[:], in0=dsl, in1=Bst[:], s0=M1_, s1=N1_)

            # ---- psi -> c -> R -> W  (all (128, 384), layout [t,ht,b]) -
            Praw = sb.tile([128, T * FW], f32, tag="Praw")     # 2*psi
            nc.scalar.activation(Praw[:], Dbuf[:], Act.Abs)
            nc.scalar.activation(Praw[:], Praw[:], Act.Relu, bias=1.0, scale=-1.0 / VTH)

            cbuf = sb.tile([128, (T + 1) * FW], f32, tag="cbuf")
            nc.vector.tensor_scalar(
                cbuf[:, : T * FW], Praw[:], -0.5 * BETA, RHO, Alu.mult, Alu.add
            )
            nc.gpsimd.memset(cbuf[:, T * FW :], 0.0)

            kaP = sb.tile([128, T * FW], f32, tag="kaP")
            nc.vector.tensor_mul(kaP[:], Praw[:], kap[:])

            Rr = sb.tile([128, T * FW], f32, tag="Rr")
            cb_ap = cbuf[:]
            ka_ap = kaP[:]
            rr_ap = Rr[:]
            for j in range(FW):
                # backward scan over t:  R_t = kaP_t + c_{t+1} R_{t+1}
                nc.vector.tensor_tensor_scan(
                    rr_ap[:, FW * (T - 1) + j :: -FW],
                    cb_ap[:, FW * T + j : j : -FW],
                    ka_ap[:, FW * (T - 1) + j :: -FW],
                    0.0,
                    Alu.mult,
                    Alu.add,
                )
            Wr = sb.tile([128, T * FW], f32, tag="Wr")
            nc.vector.scalar_tensor_tensor(
                Wr[:], in0=Rr[:], scalar=-0.5 * BETA, in1=kap[:], op0=Alu.mult, op1=Alu.add
            )
            nc.vector.tensor_mul(Wr[:], Praw[:], Wr[:])

            # ---- transpose W and Z into (t, h) layout ------------------
            Wt = sb.tile([T, BS * H], f32, tag="Wt")
            Zt = sb.tile([T, BS * H], f32, tag="Zt")
            for idx, (src, dst) in enumerate(((Wr, Wt), (Zbuf, Zt))):
                for ht in range(HT):
                    for b in range(BS):
                        tp = ps_tp.tile([T, 128], f32, tag="tp")
                        nc.tensor.transpose(
                            tp[:], src[:, BS * ht + b :: FW], ident[:]
                        )
                        dst_sl = dst[:, b * H + ht * 128 : b * H + (ht + 1) * 128]
                        if (ht + b + idx) % 2 == 0:
                            nc.scalar.copy(dst_sl, tp[:])
                        else:
                            nc.vector.tensor_copy(dst_sl, tp[:])

            # ---- TrRec = (0.5 La'') @ Z --------------------------------
            TrRec = sb.tile([T, BS * H], f32, tag="TrRec")
            for k in range(2):
                trr_ps = ps_tr.tile([T, 512], f32, tag="tr_ps")
                nc.tensor.matmul(
                    trr_ps[:], lhsT=la2T[:], rhs=Zt[:, 512 * k : 512 * (k + 1)],
                    start=True, stop=True,
                )
                nc.scalar.copy(TrRec[:, 512 * k : 512 * (k + 1)], trr_ps[:])

            # ---- readout: Y = Z @ w_out.T, kappa-scan over t -----------
            y_ps = ps_y.tile([O, T * BS], f32, tag="y_ps")
            z_v = Zbuf[:].rearrange("p (t x) -> p t x", x=FW)
            for kt in range(HT):
                nc.tensor.matmul(
                    y_ps[:],
                    lhsT=woutT[:, kt * O : (kt + 1) * O],
                    rhs=z_v[:, :, BS * kt : BS * (kt + 1)],
                    start=(kt == 0),
                    stop=(kt == HT - 1),
                )
            kconst = sb.tile([O, T], f32, tag="kconst")
            nc.gpsimd.memset(kconst[:], KAPPA)
            ro = sb.tile([O, BS * T], f32, tag="ro")
            for b in range(BS):
                nc.vector.tensor_tensor_scan(
                    ro[:, T * b : T * (b + 1)],
                    kconst[:],
                    y_ps[:, b::BS],
                    0.0,
                    Alu.mult,
                    Alu.add,
                )
            nc.sync.dma_start(o_ro_d.ap(), ro[:])

            # ---- fe_in / fe_rec: per (b, ht): W^T-slice @ [TrIn | TrRec]
            fe_sb = sb.tile([128, BS * HT * (I + H)], f32, tag="fe_sb")
            for b in range(BS):
                for ht in range(HT):
                    fe_ps = ps_fe.tile([128, I + H], f32, tag="fe_ps")
                    lhs = Wt[:, b * H + ht * 128 : b * H + (ht + 1) * 128]
                    nc.tensor.matmul(
                        fe_ps[:, :I], lhsT=lhs, rhs=TrIn[:, b * I : (b + 1) * I],
                        start=True, stop=True,
                    )
                    nc.tensor.matmul(
                        fe_ps[:, I:], lhsT=lhs, rhs=TrRec[:, b * H : (b + 1) * H],
                        start=True, stop=True,
                    )
                    dst = fe_sb[:, (b * HT + ht) * (I + H) : (b * HT + ht + 1) * (I + H)]
                    if (b + ht) % 2 == 0:
                        nc.scalar.copy(dst, fe_ps[:])
                    else:
                        nc.vector.tensor_copy(dst, fe_ps[:])
            half = BS * HT * (I + H) // 2
            nc.sync.dma_start(o_fe_d.ap()[:, :half], fe_sb[:, :half])
            nc.sync.dma_start(o_fe_d.ap()[:, half:], fe_sb[:, half:])

            if debug:
                nc.sync.dma_start(dbg_z_d.ap(), Zbuf[:])
                nc.sync.dma_start(dbg_d_d.ap(), Dbuf[:])
                nc.sync.dma_start(dbg_w_d.ap(), Wr[:])
    return nc


def prepare_in_maps(x_seq, w_in, w_rec, w_out):
    x_seq = np.ascontiguousarray(x_seq, np.float32)
    w_inT = np.ascontiguousarray(w_in.T, np.float32)
    w_recT2 = np.ascontiguousarray(
        w_rec.T.reshape(HT, 128, H).transpose(1, 0, 2).reshape(128, HT * H), np.float32
    )
    w_outT2 = np.ascontiguousarray(
        w_out.T.reshape(HT, 128, O).transpose(1, 0, 2).reshape(128, HT * O), np.float32
    )
    in_maps = []
    for c in range(NCORES):
        xc = x_seq[c * BS : (c + 1) * BS]  # (4, 48, 128)
        in_maps.append(
            {
                "x_itb": np.ascontiguousarray(xc.transpose(2, 1, 0).reshape(I, T * BS)),
                "x_tbi": np.ascontiguousarray(xc.transpose(1, 0, 2).reshape(T, BS * I)),
                "w_inT": w_inT,
                "w_recT2": w_recT2,
                "w_outT2": w_outT2,
            }
        )
    return in_maps


def assemble_outputs(results):
    readout = np.empty((B, T, O), np.float32)
    fe_in = np.empty((B, H, I), np.float32)
    fe_rec = np.empty((B, H, H), np.float32)
    for c in range(NCORES):
        o_fe = results[c]["o_fe"]  # (128, BS*HT*(I+H))
        o_ro = results[c]["o_ro"]  # (O, BS*T)
        fe = o_fe.reshape(128, BS, HT, I + H)
        for b in range(BS):
            g = c * BS + b
            readout[g] = o_ro[:, b * T : (b + 1) * T].T
            for ht in range(HT):
                fe_in[g, ht * 128 : (ht + 1) * 128, :] = fe[:, b, ht, :I]
                fe_rec[g, ht * 128 : (ht + 1) * 128, :] = fe[:, b, ht, I:]
    return readout, (fe_in, fe_rec)


_COMPILED = {}


def _get_compiled():
    if "nc" not in _COMPILED:
        import concourse.bacc as bacc

        nc = bacc.Bacc("TRN2", target_bir_lowering=False, debug=False, num_devices=NCORES)
        build(nc, debug=False)
        nc.compile()
        _COMPILED["nc"] = nc
    return _COMPILED["nc"]


def kernel(x_seq, w_in, w_rec, w_out):
    from concourse import bass_utils

    nc = _get_compiled()
    in_maps = prepare_in_maps(x_seq, w_in, w_rec, w_out)
    res = bass_utils.run_bass_kernel_spmd(nc, in_maps, core_ids=list(range(NCORES)))
    return assemble_outputs(res.results)
